# revision 9
# baseline (speedup 1.0000x reference)
"""Trainium2 Bass kernel for AdvancedKANLayer.

Math (per reference):
  xn = tanh(x)                                  (B, IN)
  d_g = |xn - g|                                for 8 grid points g
  f(d) = 2*(1-d)+^3 - 8*(0.5-d)+^3              (piecewise-cubic B-spline basis)
  out[b,o] = sum_{i,g} f(d_g[b,i]) * sw[o,i,g] + 0.1 * xn @ ba.T

Device formulation (per core, batch-sharded 8 ways, b_loc=512):
  d    = |xn - g|                 (DVE tensor_scalar: (xn-g) abs_max 0, 4x mode)
  mA   = min(d-1,   0) = -(1-d)+      (DVE tensor_scalar, 4x)
  mB   = min(d-0.5, 0) = -(0.5-d)+    (DVE tensor_scalar, 4x)
  sA   = Square(sqrt(2)*mA) = 2*(1-d)+^2     (ACT, scale folded into Square)
  sB   = Square(sqrt(8)*mB) = 8*(0.5-d)+^2   (ACT)
  cA   = sA*mA = -2*(1-d)+^3      \  one tensor_tensor over both halves (2x)
  cB   = sB*mB = -8*(0.5-d)+^3    /
  F    = cB - cA = f              (DVE tensor_tensor, 2x)
  out  = W2.T @ [F channels (8 per i-tile), xn channel]  -- single PE contraction,
         K = 4*(8+1)*128 = 4608. W2 = [sw | 0.1*ba] with no scale folds.

Layout: i on partitions (4 tiles of 128), b on free dim (512). x is passed
transposed per core: xT[i, b]. Output is [o, b] per core, gathered + transposed
on host. All elementwise tensors fp16; matmul fp16 x fp16 -> fp32 PSUM.
"""

import sys

if "/opt/trn_rl_repo" not in sys.path:
    sys.path.insert(0, "/opt/trn_rl_repo")

import numpy as np

IN_F = 512
OUT_F = 512
GRID = 8
BATCH = 4096
NCORES = 8
B_LOC = BATCH // NCORES  # 512
NT = IN_F // 128         # 4 i-tiles
NO = OUT_F // 128        # 4 o-tiles
NCH = GRID + 1           # 8 basis channels + 1 xn channel per i-tile
NK = NT * NCH            # 36 k-tiles of 128

CFG = {
    "k_abs": 2,          # g's 0..k_abs-1 use ACT Abs route; rest use DVE min-of-relus
    "sq_on_act": 2,      # 0..2 of the two Square layers on ScalarE (rest DVE stt)
    "copy_on_act": True, # PSUM->SBUF output copies on ScalarE
}

_CACHE = {}

SQRT2 = float(np.sqrt(2.0))
SQRT8 = float(np.sqrt(8.0))


def _build(grid_vals, cfg):
    import concourse.tile as tile
    import concourse.mybir as mybir
    from concourse import bacc

    dt = mybir.dt
    f16 = dt.float16
    f32 = dt.float32
    AF = mybir.ActivationFunctionType
    OP = mybir.AluOpType

    nc = bacc.Bacc("TRN2", target_bir_lowering=False, debug=False)
    xT = nc.dram_tensor("xT", [IN_F, B_LOC], f32, kind="ExternalInput")
    w2 = nc.dram_tensor("w2", [NK * 128, OUT_F], f16, kind="ExternalInput")
    out = nc.dram_tensor("out", [OUT_F, B_LOC], f32, kind="ExternalOutput")

    GB = GRID * B_LOC  # 4096

    with tile.TileContext(nc) as tc:
        with (
            tc.tile_pool(name="consts", bufs=1) as cpool,
            tc.tile_pool(name="w", bufs=1) as wpool,
            tc.tile_pool(name="x", bufs=2) as xpool,
            tc.tile_pool(name="elem", bufs=2) as epool,
            tc.tile_pool(name="fch", bufs=2) as fpool,
            tc.tile_pool(name="osb", bufs=2) as opool,
            tc.tile_pool(name="ps", bufs=1, space="PSUM") as pspool,
        ):
            # Per-partition bias constants -g for optional ACT Abs ops.
            gbias = cpool.tile([128, GRID], f32)
            for g in range(GRID):
                nc.vector.memset(gbias[:, g : g + 1], -float(grid_vals[g]))

            # Extended weights: one tile per k-tile so matmuls only wait on
            # their own DMA.
            w2ap = w2.ap().rearrange("(n p) o -> n p o", p=128)
            wtiles = []
            for kt in range(NK):
                wt = wpool.tile([128, OUT_F], f16, tag=f"w{kt}", name=f"w{kt}")
                nc.sync.dma_start(out=wt[:], in_=w2ap[kt])
                wtiles.append(wt)

            psums = [
                pspool.tile([128, B_LOC], f32, tag=f"ps{ot}", name=f"ps{ot}")
                for ot in range(NO)
            ]

            k_abs = cfg["k_abs"]
            KB = k_abs * B_LOC          # abs-route span in the g*b free dim
            xTap = xT.ap().rearrange("(t p) b -> t p b", p=128)
            for t in range(NT):
                xt32 = xpool.tile([128, B_LOC], f32, tag="xt32")
                nc.sync.dma_start(out=xt32[:], in_=xTap[t])
                xn = xpool.tile([128, B_LOC], f16, tag="xn")
                nc.scalar.activation(xn[:], xt32[:], AF.Tanh)

                # xn-channel matmuls first: they only need xn, keeping PE warm
                # while the basis channels are still being computed.
                for ot in range(NO):
                    nc.tensor.matmul(
                        psums[ot][:],
                        wtiles[t * NCH + GRID][:, ot * 128 : (ot + 1) * 128],
                        xn[:],
                        start=(t == 0),
                        stop=False,
                    )

                # M = [A-chain || B-chain], each [128, GB].
                #  abs-route g (< k_abs):  A-slot = -(1-d)+,  B-slot = -(a-0.5)+
                #  min-route g (>= k_abs): A-slot = +(1-d)+,  B-slot = +(a-0.5)+
                # (sign difference is absorbed into W2's per-channel sign)
                M = epool.tile([128, 2 * GB], f16, tag="M")

                if k_abs > 0:
                    D = epool.tile([128, KB], f16, tag="D")
                    for g in range(k_abs):
                        nc.scalar.activation(
                            D[:, g * B_LOC : (g + 1) * B_LOC], xn[:], AF.Abs,
                            bias=gbias[:, g : g + 1], scale=1.0,
                        )
                    # mA = min(d-1, 0) = -(1-d)+
                    nc.vector.tensor_scalar(
                        M[:, :KB], D[:], 1.0, 0.0, OP.subtract, OP.min
                    )
                    # y' = min(mA+0.5, 0) = -(a-0.5)+
                    nc.vector.tensor_scalar(
                        M[:, GB : GB + KB], M[:, :KB], 0.5, 0.0, OP.add, OP.min
                    )
                if k_abs < GRID:
                    nxn = xpool.tile([128, B_LOC], f16, tag="nxn")
                    nc.vector.tensor_scalar(nxn[:], xn[:], -1.0, None, OP.mult)
                    R1 = epool.tile([128, GB - KB], f16, tag="R1")
                    R2 = epool.tile([128, GB - KB], f16, tag="R2")
                    for g in range(k_abs, GRID):
                        gv = float(grid_vals[g])
                        sl = slice((g - k_abs) * B_LOC, (g - k_abs + 1) * B_LOC)
                        # r1 = relu(1 - u) = relu(-xn + g + 1)
                        nc.vector.tensor_scalar(
                            R1[:, sl], nxn[:], -(gv + 1.0), 0.0, OP.subtract, OP.max
                        )
                        # r2 = relu(1 + u) = relu(xn - g + 1)
                        nc.vector.tensor_scalar(
                            R2[:, sl], xn[:], gv - 1.0, 0.0, OP.subtract, OP.max
                        )
                    # a = min(r1, r2) = (1-|u|)+
                    nc.vector.tensor_tensor(M[:, KB:GB], R1[:], R2[:], OP.min)
                    # b = relu(a - 0.5)
                    nc.vector.tensor_scalar(
                        M[:, GB + KB :], M[:, KB:GB], 0.5, 0.0, OP.subtract, OP.max
                    )

                SQ = epool.tile([128, 2 * GB], f16, tag="SQ")
                if cfg["sq_on_act"] >= 1:
                    nc.scalar.activation(SQ[:, :GB], M[:, :GB], AF.Square, scale=SQRT2)
                else:
                    nc.vector.scalar_tensor_tensor(
                        SQ[:, :GB], M[:, :GB], 2.0, M[:, :GB], OP.mult, OP.mult
                    )
                if cfg["sq_on_act"] >= 2:
                    nc.scalar.activation(SQ[:, GB:], M[:, GB:], AF.Square, scale=SQRT8)
                else:
                    nc.vector.scalar_tensor_tensor(
                        SQ[:, GB:], M[:, GB:], 8.0, M[:, GB:], OP.mult, OP.mult
                    )

                # CU_A = 2a^3 (sign follows A-slot), CU_B = 8b^3
                CU = epool.tile([128, 2 * GB], f16, tag="CU")
                nc.vector.tensor_tensor(CU[:], SQ[:], M[:], OP.mult)

                # F = CU_A - CU_B = +-(2a^3 - 8b^3) = +-f
                F = fpool.tile([128, GB], f16, tag="F")
                nc.vector.tensor_tensor(F[:], CU[:, :GB], CU[:, GB:], OP.subtract)

                for ot in range(NO):
                    for ch in range(GRID):
                        rhs = F[:, ch * B_LOC : (ch + 1) * B_LOC]
                        kt = t * NCH + ch
                        nc.tensor.matmul(
                            psums[ot][:],
                            wtiles[kt][:, ot * 128 : (ot + 1) * 128],
                            rhs,
                            start=False,
                            stop=(t == NT - 1 and ch == GRID - 1),
                        )

            for ot in range(NO):
                osb = opool.tile([128, B_LOC], f32, tag="osb")
                if cfg["copy_on_act"]:
                    nc.scalar.copy(osb[:], psums[ot][:])
                else:
                    nc.vector.tensor_copy(osb[:], psums[ot][:])
                nc.sync.dma_start(
                    out=out.ap()[ot * 128 : (ot + 1) * 128, :], in_=osb[:]
                )

    nc.compile()
    return nc


def _get_nc(grid_vals, cfg=None):
    cfg = cfg or CFG
    key = (tuple(np.asarray(grid_vals, np.float32).tolist()), tuple(sorted(cfg.items())))
    if key not in _CACHE:
        _CACHE[key] = _build(grid_vals, cfg)
    return _CACHE[key]


def _prep_inputs(x, spline_weight, base_activation, k_abs):
    x = np.asarray(x, np.float32)
    sw = np.asarray(spline_weight, np.float32)
    ba = np.asarray(base_activation, np.float32)
    # W2[k, o] with k = (t*NCH + ch)*128 + p ; ch<8 -> +-sw[o, i, g] ; ch==8 -> 0.1*ba[o, i]
    # abs-route channels (g < k_abs) produce -f, so their weights are negated.
    W2 = np.empty((NK, 128, OUT_F), np.float32)
    sw_t = sw.transpose(1, 2, 0)  # [in, g, out]
    ba_t = ba.T  # [in, out]
    for t in range(NT):
        isl = slice(t * 128, (t + 1) * 128)
        for g in range(GRID):
            sgn = -1.0 if g < k_abs else 1.0
            W2[t * NCH + g] = sgn * sw_t[isl, g, :]
        W2[t * NCH + GRID] = 0.1 * ba_t[isl, :]
    W2 = W2.reshape(NK * 128, OUT_F).astype(np.float16)
    xT = np.ascontiguousarray(x.T)  # [IN_F, BATCH]
    in_maps = [
        {
            "xT": np.ascontiguousarray(xT[:, c * B_LOC : (c + 1) * B_LOC]),
            "w2": W2,
        }
        for c in range(NCORES)
    ]
    return in_maps


def _run(x, spline_weight, base_activation, grid_points, trace=False, cfg=None,
         tmpdir=None):
    from concourse.bass_utils import run_bass_kernel_spmd

    nc = _get_nc(np.asarray(grid_points, np.float32), cfg)
    in_maps = _prep_inputs(x, spline_weight, base_activation, (cfg or CFG)["k_abs"])
    res = run_bass_kernel_spmd(
        nc, in_maps, list(range(NCORES)), trace=trace, tmpdir=tmpdir
    )
    outs = [res.results[c]["out"] for c in range(NCORES)]  # each [OUT_F, B_LOC]
    full = np.concatenate(outs, axis=1)  # [OUT_F, BATCH]
    return np.ascontiguousarray(full.T.astype(np.float32)), res


def kernel(x, spline_weight, base_activation, grid_points):
    out, _ = _run(x, spline_weight, base_activation, grid_points)
    return out


# revision 16
# speedup vs baseline: 1.3043x; 1.3043x over previous
"""Trainium2 Bass kernel for AdvancedKANLayer.

Math (per reference):
  xn = tanh(x)                                  (B, IN)
  d_g = |xn - g|                                for 8 grid points g
  f(d) = 2*(1-d)+^3 - 8*(0.5-d)+^3              (piecewise-cubic B-spline basis)
  out[b,o] = sum_{i,g} f(d_g[b,i]) * sw[o,i,g] + 0.1 * xn @ ba.T

Device formulation (per core, batch-sharded 8 ways, b_loc=512):
  d    = |xn - g|                 (DVE tensor_scalar: (xn-g) abs_max 0, 4x mode)
  mA   = min(d-1,   0) = -(1-d)+      (DVE tensor_scalar, 4x)
  mB   = min(d-0.5, 0) = -(0.5-d)+    (DVE tensor_scalar, 4x)
  sA   = Square(sqrt(2)*mA) = 2*(1-d)+^2     (ACT, scale folded into Square)
  sB   = Square(sqrt(8)*mB) = 8*(0.5-d)+^2   (ACT)
  cA   = sA*mA = -2*(1-d)+^3      \  one tensor_tensor over both halves (2x)
  cB   = sB*mB = -8*(0.5-d)+^3    /
  F    = cB - cA = f              (DVE tensor_tensor, 2x)
  out  = W2.T @ [F channels (8 per i-tile), xn channel]  -- single PE contraction,
         K = 4*(8+1)*128 = 4608. W2 = [sw | 0.1*ba] with no scale folds.

Layout: i on partitions (4 tiles of 128), b on free dim (512). x is passed
transposed per core: xT[i, b]. Output is [o, b] per core, gathered + transposed
on host. All elementwise tensors fp16; matmul fp16 x fp16 -> fp32 PSUM.
"""

import sys

if "/opt/trn_rl_repo" not in sys.path:
    sys.path.insert(0, "/opt/trn_rl_repo")

import numpy as np

IN_F = 512
OUT_F = 512
GRID = 8
BATCH = 4096
NCORES = 8
B_LOC = BATCH // NCORES  # 512
NT = IN_F // 128         # 4 i-tiles
NO = OUT_F // 128        # 4 o-tiles
NCH = GRID + 1           # 8 basis channels + 1 xn channel per i-tile
NK = NT * NCH            # 36 k-tiles of 128

CFG = {
    "k_abs": 4,          # g's 0..k_abs-1 use ACT Abs route; rest use DVE min-of-relus
    "sq_on_act": 2,      # 0..2 of the two Square layers on ScalarE (rest DVE stt)
    "copy_on_act": True, # PSUM->SBUF output copies on ScalarE
}

# Weight k-tiles are DMA'd packed PACK-at-a-time (PACK*1KB contiguous DRAM rows)
# in the order the matmuls consume them: per i-tile, the xn channel first.
PACK = 4
CONSUME_ORDER = []
for _t in range(NT):
    CONSUME_ORDER.append(_t * NCH + GRID)
    CONSUME_ORDER.extend(_t * NCH + _g for _g in range(GRID))
KT_SLOT = {kt: (j // PACK, j % PACK) for j, kt in enumerate(CONSUME_ORDER)}

_CACHE = {}

SQRT2 = float(np.sqrt(2.0))
SQRT8 = float(np.sqrt(8.0))


def _build(grid_vals, cfg):
    import concourse.tile as tile
    import concourse.mybir as mybir
    from concourse import bacc

    dt = mybir.dt
    f16 = dt.float16
    f32 = dt.float32
    AF = mybir.ActivationFunctionType
    OP = mybir.AluOpType

    nc = bacc.Bacc("TRN2", target_bir_lowering=False, debug=False)
    xT = nc.dram_tensor("xT", [IN_F, B_LOC], f32, kind="ExternalInput")
    w2 = nc.dram_tensor("w2", [NK // PACK * 128, PACK * OUT_F], f16,
                        kind="ExternalInput")
    out = nc.dram_tensor("out", [OUT_F, B_LOC], f32, kind="ExternalOutput")

    GB = GRID * B_LOC  # 4096

    with tile.TileContext(nc) as tc:
        with (
            tc.tile_pool(name="consts", bufs=1) as cpool,
            tc.tile_pool(name="w", bufs=1) as wpool,
            tc.tile_pool(name="x", bufs=2) as xpool,
            tc.tile_pool(name="elem", bufs=2) as epool,
            tc.tile_pool(name="fch", bufs=2) as fpool,
            tc.tile_pool(name="osb", bufs=2) as opool,
            tc.tile_pool(name="ps", bufs=1, space="PSUM") as pspool,
        ):
            k_abs = cfg["k_abs"]
            KB = k_abs * B_LOC          # abs-route span in the g*b free dim

            # Input x tiles first -- nothing can start until these land.
            xTap = xT.ap().rearrange("(t p) b -> t p b", p=128)
            xt32s = []
            for t in range(NT):
                xt32 = xpool.tile([128, B_LOC], f32, tag=f"xt32_{t}",
                                  name=f"xt32_{t}")
                nc.sync.dma_start(out=xt32[:], in_=xTap[t])
                xt32s.append(xt32)

            # Per-partition bias constants -g for optional ACT Abs ops.
            if k_abs > 0:
                gbias = cpool.tile([128, max(k_abs, 1)], f32)
                for g in range(k_abs):
                    nc.vector.memset(gbias[:, g : g + 1], -float(grid_vals[g]))

            # Weights, PACK k-tiles per DMA, in matmul consumption order.
            w2ap = w2.ap().rearrange("(n p) o -> n p o", p=128)
            wslabs = []
            for j in range(NK // PACK):
                ws = wpool.tile([128, PACK * OUT_F], f16, tag=f"w{j}", name=f"w{j}")
                nc.sync.dma_start(out=ws[:], in_=w2ap[j])
                wslabs.append(ws)

            def wslice(kt, ot):
                j, h = KT_SLOT[kt]
                base = h * OUT_F + ot * 128
                return wslabs[j][:, base : base + 128]

            psums = [
                pspool.tile([128, B_LOC], f32, tag=f"ps{ot}", name=f"ps{ot}")
                for ot in range(NO)
            ]

            for t in range(NT):
                xn = xpool.tile([128, B_LOC], f16, tag=f"xn{t}", name=f"xn{t}")
                nc.scalar.activation(xn[:], xt32s[t][:], AF.Tanh)

                # xn-channel matmuls first: they only need xn, keeping PE warm
                # while the basis channels are still being computed.
                for ot in range(NO):
                    nc.tensor.matmul(
                        psums[ot][:],
                        wslice(t * NCH + GRID, ot),
                        xn[:],
                        start=(t == 0),
                        stop=False,
                    )

                # M = [A-chain || B-chain], each [128, GB].
                #  abs-route g (< k_abs):  A-slot = -(1-d)+,  B-slot = -(a-0.5)+
                #  min-route g (>= k_abs): A-slot = +(1-d)+,  B-slot = +(a-0.5)+
                # (sign difference is absorbed into W2's per-channel sign)
                M = epool.tile([128, 2 * GB], f16, tag="M")

                if k_abs > 0:
                    D = epool.tile([128, KB], f16, tag="D")
                    for g in range(k_abs):
                        nc.scalar.activation(
                            D[:, g * B_LOC : (g + 1) * B_LOC], xn[:], AF.Abs,
                            bias=gbias[:, g : g + 1], scale=1.0,
                        )
                    # mA = min(d-1, 0) = -(1-d)+
                    nc.vector.tensor_scalar(
                        M[:, :KB], D[:], 1.0, 0.0, OP.subtract, OP.min
                    )
                    # y' = min(mA+0.5, 0) = -(a-0.5)+
                    nc.vector.tensor_scalar(
                        M[:, GB : GB + KB], M[:, :KB], 0.5, 0.0, OP.add, OP.min
                    )
                if k_abs < GRID:
                    nxn = xpool.tile([128, B_LOC], f16, tag="nxn")
                    nc.vector.tensor_scalar(nxn[:], xn[:], -1.0, None, OP.mult)
                    R1 = epool.tile([128, GB - KB], f16, tag="R1")
                    R2 = epool.tile([128, GB - KB], f16, tag="R2")
                    for g in range(k_abs, GRID):
                        gv = float(grid_vals[g])
                        sl = slice((g - k_abs) * B_LOC, (g - k_abs + 1) * B_LOC)
                        # r1 = relu(1 - u) = relu(-xn + g + 1)
                        nc.vector.tensor_scalar(
                            R1[:, sl], nxn[:], -(gv + 1.0), 0.0, OP.subtract, OP.max
                        )
                        # r2 = relu(1 + u) = relu(xn - g + 1)
                        nc.vector.tensor_scalar(
                            R2[:, sl], xn[:], gv - 1.0, 0.0, OP.subtract, OP.max
                        )
                    # a = min(r1, r2) = (1-|u|)+
                    nc.vector.tensor_tensor(M[:, KB:GB], R1[:], R2[:], OP.min)
                    # b = relu(a - 0.5)
                    nc.vector.tensor_scalar(
                        M[:, GB + KB :], M[:, KB:GB], 0.5, 0.0, OP.subtract, OP.max
                    )

                SQ = epool.tile([128, 2 * GB], f16, tag="SQ")
                if cfg["sq_on_act"] >= 1:
                    nc.scalar.activation(SQ[:, :GB], M[:, :GB], AF.Square, scale=SQRT2)
                else:
                    nc.vector.scalar_tensor_tensor(
                        SQ[:, :GB], M[:, :GB], 2.0, M[:, :GB], OP.mult, OP.mult
                    )
                if cfg["sq_on_act"] >= 2:
                    nc.scalar.activation(SQ[:, GB:], M[:, GB:], AF.Square, scale=SQRT8)
                else:
                    nc.vector.scalar_tensor_tensor(
                        SQ[:, GB:], M[:, GB:], 8.0, M[:, GB:], OP.mult, OP.mult
                    )

                # CU_A = 2a^3 (sign follows A-slot), CU_B = 8b^3
                CU = epool.tile([128, 2 * GB], f16, tag="CU")
                nc.vector.tensor_tensor(CU[:], SQ[:], M[:], OP.mult)

                # F = CU_A - CU_B = +-(2a^3 - 8b^3) = +-f
                F = fpool.tile([128, GB], f16, tag="F")
                nc.vector.tensor_tensor(F[:], CU[:, :GB], CU[:, GB:], OP.subtract)

                for ot in range(NO):
                    for ch in range(GRID):
                        rhs = F[:, ch * B_LOC : (ch + 1) * B_LOC]
                        kt = t * NCH + ch
                        nc.tensor.matmul(
                            psums[ot][:],
                            wslice(kt, ot),
                            rhs,
                            start=False,
                            stop=(t == NT - 1 and ch == GRID - 1),
                        )

            for ot in range(NO):
                osb = opool.tile([128, B_LOC], f32, tag="osb")
                if cfg["copy_on_act"]:
                    nc.scalar.copy(osb[:], psums[ot][:])
                else:
                    nc.vector.tensor_copy(osb[:], psums[ot][:])
                nc.sync.dma_start(
                    out=out.ap()[ot * 128 : (ot + 1) * 128, :], in_=osb[:]
                )

    nc.compile()
    return nc


def _get_nc(grid_vals, cfg=None):
    cfg = cfg or CFG
    key = (tuple(np.asarray(grid_vals, np.float32).tolist()), tuple(sorted(cfg.items())))
    if key not in _CACHE:
        _CACHE[key] = _build(grid_vals, cfg)
    return _CACHE[key]


def _prep_inputs(x, spline_weight, base_activation, k_abs):
    x = np.asarray(x, np.float32)
    sw = np.asarray(spline_weight, np.float32)
    ba = np.asarray(base_activation, np.float32)
    # W2[k, o] with k = (t*NCH + ch)*128 + p ; ch<8 -> +-sw[o, i, g] ; ch==8 -> 0.1*ba[o, i]
    # abs-route channels (g < k_abs) produce -f, so their weights are negated.
    W2 = np.empty((NK, 128, OUT_F), np.float32)
    sw_t = sw.transpose(1, 2, 0)  # [in, g, out]
    ba_t = ba.T  # [in, out]
    for t in range(NT):
        isl = slice(t * 128, (t + 1) * 128)
        for g in range(GRID):
            sgn = -1.0 if g < k_abs else 1.0
            W2[t * NCH + g] = sgn * sw_t[isl, g, :]
        W2[t * NCH + GRID] = 0.1 * ba_t[isl, :]
    # Pack PACK k-tiles per DMA slab, in matmul consumption order:
    # slab j, partition p holds [W2[ord[j*PACK+h]][p] for h in 0..PACK-1].
    W2p = np.empty((NK // PACK, 128, PACK * OUT_F), np.float32)
    for j in range(NK // PACK):
        for h in range(PACK):
            W2p[j, :, h * OUT_F : (h + 1) * OUT_F] = W2[CONSUME_ORDER[j * PACK + h]]
    W2 = W2p.reshape(NK // PACK * 128, PACK * OUT_F).astype(np.float16)
    xT = np.ascontiguousarray(x.T)  # [IN_F, BATCH]
    in_maps = [
        {
            "xT": np.ascontiguousarray(xT[:, c * B_LOC : (c + 1) * B_LOC]),
            "w2": W2,
        }
        for c in range(NCORES)
    ]
    return in_maps


def _run(x, spline_weight, base_activation, grid_points, trace=False, cfg=None,
         tmpdir=None):
    from concourse.bass_utils import run_bass_kernel_spmd

    nc = _get_nc(np.asarray(grid_points, np.float32), cfg)
    in_maps = _prep_inputs(x, spline_weight, base_activation, (cfg or CFG)["k_abs"])
    res = run_bass_kernel_spmd(
        nc, in_maps, list(range(NCORES)), trace=trace, tmpdir=tmpdir
    )
    outs = [res.results[c]["out"] for c in range(NCORES)]  # each [OUT_F, B_LOC]
    full = np.concatenate(outs, axis=1)  # [OUT_F, BATCH]
    return np.ascontiguousarray(full.T.astype(np.float32)), res


def kernel(x, spline_weight, base_activation, grid_points):
    out, _ = _run(x, spline_weight, base_activation, grid_points)
    return out


# revision 18
# speedup vs baseline: 1.7530x; 1.3440x over previous
"""Trainium2 Bass kernel for AdvancedKANLayer.

Math (per reference):
  xn = tanh(x)                                  (B, IN)
  d_g = |xn - g|                                for 8 grid points g
  f(d) = 2*(1-d)+^3 - 8*(0.5-d)+^3              (piecewise-cubic B-spline basis)
  out[b,o] = sum_{i,g} f(d_g[b,i]) * sw[o,i,g] + 0.1 * xn @ ba.T

Device formulation (per core, batch-sharded 8 ways, b_loc=512):
  d    = |xn - g|                 (DVE tensor_scalar: (xn-g) abs_max 0, 4x mode)
  mA   = min(d-1,   0) = -(1-d)+      (DVE tensor_scalar, 4x)
  mB   = min(d-0.5, 0) = -(0.5-d)+    (DVE tensor_scalar, 4x)
  sA   = Square(sqrt(2)*mA) = 2*(1-d)+^2     (ACT, scale folded into Square)
  sB   = Square(sqrt(8)*mB) = 8*(0.5-d)+^2   (ACT)
  cA   = sA*mA = -2*(1-d)+^3      \  one tensor_tensor over both halves (2x)
  cB   = sB*mB = -8*(0.5-d)+^3    /
  F    = cB - cA = f              (DVE tensor_tensor, 2x)
  out  = W2.T @ [F channels (8 per i-tile), xn channel]  -- single PE contraction,
         K = 4*(8+1)*128 = 4608. W2 = [sw | 0.1*ba] with no scale folds.

Layout: i on partitions (4 tiles of 128), b on free dim (512). x is passed
transposed per core: xT[i, b]. Output is [o, b] per core, gathered + transposed
on host. All elementwise tensors fp16; matmul fp16 x fp16 -> fp32 PSUM.
"""

import sys

if "/opt/trn_rl_repo" not in sys.path:
    sys.path.insert(0, "/opt/trn_rl_repo")

import numpy as np

IN_F = 512
OUT_F = 512
GRID = 8
BATCH = 4096
NCORES = 8
B_LOC = BATCH // NCORES  # 512
NT = IN_F // 128         # 4 i-tiles
NO = OUT_F // 128        # 4 o-tiles
NCH = GRID + 1           # 8 basis channels + 1 xn channel per i-tile
NK = NT * NCH            # 36 k-tiles of 128

CFG = {
    "k_abs": 4,          # g's 0..k_abs-1 use ACT Abs route; rest use DVE min-of-relus
    "sq_on_act": 2,      # 0..2 of the two Square layers on ScalarE (rest DVE stt)
    "copy_on_act": True, # PSUM->SBUF output copies on ScalarE
}

# Weight k-tiles are DMA'd packed PACK-at-a-time (PACK*1KB contiguous DRAM rows)
# in the order the matmuls consume them: per i-tile, the xn channel first.
PACK = 4
CONSUME_ORDER = []
for _t in range(NT):
    CONSUME_ORDER.append(_t * NCH + GRID)
    CONSUME_ORDER.extend(_t * NCH + _g for _g in range(GRID))
KT_SLOT = {kt: (j // PACK, j % PACK) for j, kt in enumerate(CONSUME_ORDER)}

_CACHE = {}

SQRT2 = float(np.sqrt(2.0))
SQRT8 = float(np.sqrt(8.0))


def _build(grid_vals, cfg):
    import concourse.tile as tile
    import concourse.mybir as mybir
    from concourse import bacc

    dt = mybir.dt
    f16 = dt.float16
    f32 = dt.float32
    AF = mybir.ActivationFunctionType
    OP = mybir.AluOpType

    nc = bacc.Bacc("TRN2", target_bir_lowering=False, debug=False)
    xT = nc.dram_tensor("xT", [IN_F, B_LOC], f32, kind="ExternalInput")
    w2 = nc.dram_tensor("w2", [NK // PACK * 128, PACK * OUT_F], f16,
                        kind="ExternalInput")
    out = nc.dram_tensor("out", [OUT_F, B_LOC], f32, kind="ExternalOutput")

    GB = GRID * B_LOC  # 4096

    with tile.TileContext(nc) as tc:
        with (
            tc.tile_pool(name="consts", bufs=1) as cpool,
            tc.tile_pool(name="w", bufs=1) as wpool,
            tc.tile_pool(name="x", bufs=2) as xpool,
            tc.tile_pool(name="elem", bufs=3) as epool,
            tc.tile_pool(name="fch", bufs=3) as fpool,
            tc.tile_pool(name="osb", bufs=2) as opool,
            tc.tile_pool(name="ps", bufs=1, space="PSUM") as pspool,
        ):
            k_abs = cfg["k_abs"]
            KB = k_abs * B_LOC          # abs-route span in the g*b free dim

            # Input x tiles first -- nothing can start until these land.
            xTap = xT.ap().rearrange("(t p) b -> t p b", p=128)
            xt32s = []
            for t in range(NT):
                xt32 = xpool.tile([128, B_LOC], f32, tag=f"xt32_{t}",
                                  name=f"xt32_{t}")
                nc.sync.dma_start(out=xt32[:], in_=xTap[t])
                xt32s.append(xt32)

            # Per-partition bias constants -g for optional ACT Abs ops.
            if k_abs > 0:
                gbias = cpool.tile([128, max(k_abs, 1)], f32)
                for g in range(k_abs):
                    nc.vector.memset(gbias[:, g : g + 1], -float(grid_vals[g]))

            # Weights, PACK k-tiles per DMA, in matmul consumption order.
            w2ap = w2.ap().rearrange("(n p) o -> n p o", p=128)
            wslabs = []
            for j in range(NK // PACK):
                ws = wpool.tile([128, PACK * OUT_F], f16, tag=f"w{j}", name=f"w{j}")
                nc.sync.dma_start(out=ws[:], in_=w2ap[j])
                wslabs.append(ws)

            def wslice(kt, ot):
                j, h = KT_SLOT[kt]
                base = h * OUT_F + ot * 128
                return wslabs[j][:, base : base + 128]

            psums = [
                pspool.tile([128, B_LOC], f32, tag=f"ps{ot}", name=f"ps{ot}")
                for ot in range(NO)
            ]

            HG = GRID // 2            # 4 g's per half
            HB = HG * B_LOC           # 2048
            for t in range(NT):
                xn = xpool.tile([128, B_LOC], f16, tag=f"xn{t}", name=f"xn{t}")
                nc.scalar.activation(xn[:], xt32s[t][:], AF.Tanh)

                # xn-channel matmuls first: they only need xn, keeping PE warm
                # while the basis channels are still being computed.
                for ot in range(NO):
                    nc.tensor.matmul(
                        psums[ot][:],
                        wslice(t * NCH + GRID, ot),
                        xn[:],
                        start=(t == 0),
                        stop=False,
                    )

                # Two g-halves per i-tile for finer-grained PE feeding.
                # half 0 (g 0..3): ACT Abs route -> A-slot = -(1-d)+, B = -(a-.5)+
                # half 1 (g 4..7): DVE min-of-relus -> A-slot = +(1-d)+, B = +(a-.5)+
                # (sign difference absorbed into W2's per-channel sign; k_abs=4)
                for h in range(2):
                    g0 = h * HG
                    M = epool.tile([128, 2 * HB], f16, tag="M")
                    if h == 0:
                        D = epool.tile([128, HB], f16, tag="D")
                        for g in range(HG):
                            nc.scalar.activation(
                                D[:, g * B_LOC : (g + 1) * B_LOC], xn[:], AF.Abs,
                                bias=gbias[:, g : g + 1], scale=1.0,
                            )
                        # mA = min(d-1, 0) = -(1-d)+
                        nc.vector.tensor_scalar(
                            M[:, :HB], D[:], 1.0, 0.0, OP.subtract, OP.min
                        )
                        # y' = min(mA+0.5, 0) = -(a-0.5)+
                        nc.vector.tensor_scalar(
                            M[:, HB:], M[:, :HB], 0.5, 0.0, OP.add, OP.min
                        )
                    else:
                        nxn = xpool.tile([128, B_LOC], f16, tag=f"nxn{t}",
                                         name=f"nxn{t}")
                        nc.vector.tensor_scalar(nxn[:], xn[:], -1.0, None, OP.mult)
                        R1 = epool.tile([128, HB], f16, tag="R1")
                        R2 = epool.tile([128, HB], f16, tag="R2")
                        for g in range(g0, GRID):
                            gv = float(grid_vals[g])
                            sl = slice((g - g0) * B_LOC, (g - g0 + 1) * B_LOC)
                            # r1 = relu(1 - u) = relu(-xn + g + 1)
                            nc.vector.tensor_scalar(
                                R1[:, sl], nxn[:], -(gv + 1.0), 0.0,
                                OP.subtract, OP.max,
                            )
                            # r2 = relu(1 + u) = relu(xn - g + 1)
                            nc.vector.tensor_scalar(
                                R2[:, sl], xn[:], gv - 1.0, 0.0,
                                OP.subtract, OP.max,
                            )
                        # a = min(r1, r2) = (1-|u|)+
                        nc.vector.tensor_tensor(M[:, :HB], R1[:], R2[:], OP.min)
                        # b = relu(a - 0.5)
                        nc.vector.tensor_scalar(
                            M[:, HB:], M[:, :HB], 0.5, 0.0, OP.subtract, OP.max
                        )

                    SQ = epool.tile([128, 2 * HB], f16, tag="SQ")
                    nc.scalar.activation(SQ[:, :HB], M[:, :HB], AF.Square,
                                         scale=SQRT2)
                    nc.scalar.activation(SQ[:, HB:], M[:, HB:], AF.Square,
                                         scale=SQRT8)

                    # CU_A = 2a^3 (sign follows A-slot), CU_B = 8b^3
                    CU = epool.tile([128, 2 * HB], f16, tag="CU")
                    nc.vector.tensor_tensor(CU[:], SQ[:], M[:], OP.mult)

                    # F = CU_A - CU_B = +-(2a^3 - 8b^3) = +-f
                    F = fpool.tile([128, HB], f16, tag="F")
                    nc.vector.tensor_tensor(F[:], CU[:, :HB], CU[:, HB:],
                                            OP.subtract)

                    for ot in range(NO):
                        for gg in range(HG):
                            ch = g0 + gg
                            rhs = F[:, gg * B_LOC : (gg + 1) * B_LOC]
                            kt = t * NCH + ch
                            nc.tensor.matmul(
                                psums[ot][:],
                                wslice(kt, ot),
                                rhs,
                                start=False,
                                stop=(t == NT - 1 and ch == GRID - 1),
                            )

            for ot in range(NO):
                osb = opool.tile([128, B_LOC], f32, tag="osb")
                if cfg["copy_on_act"]:
                    nc.scalar.copy(osb[:], psums[ot][:])
                else:
                    nc.vector.tensor_copy(osb[:], psums[ot][:])
                nc.sync.dma_start(
                    out=out.ap()[ot * 128 : (ot + 1) * 128, :], in_=osb[:]
                )

    nc.compile()
    return nc


def _get_nc(grid_vals, cfg=None):
    cfg = cfg or CFG
    key = (tuple(np.asarray(grid_vals, np.float32).tolist()), tuple(sorted(cfg.items())))
    if key not in _CACHE:
        _CACHE[key] = _build(grid_vals, cfg)
    return _CACHE[key]


def _prep_inputs(x, spline_weight, base_activation, k_abs):
    x = np.asarray(x, np.float32)
    sw = np.asarray(spline_weight, np.float32)
    ba = np.asarray(base_activation, np.float32)
    # W2[k, o] with k = (t*NCH + ch)*128 + p ; ch<8 -> +-sw[o, i, g] ; ch==8 -> 0.1*ba[o, i]
    # abs-route channels (g < k_abs) produce -f, so their weights are negated.
    W2 = np.empty((NK, 128, OUT_F), np.float32)
    sw_t = sw.transpose(1, 2, 0)  # [in, g, out]
    ba_t = ba.T  # [in, out]
    for t in range(NT):
        isl = slice(t * 128, (t + 1) * 128)
        for g in range(GRID):
            sgn = -1.0 if g < k_abs else 1.0
            W2[t * NCH + g] = sgn * sw_t[isl, g, :]
        W2[t * NCH + GRID] = 0.1 * ba_t[isl, :]
    # Pack PACK k-tiles per DMA slab, in matmul consumption order:
    # slab j, partition p holds [W2[ord[j*PACK+h]][p] for h in 0..PACK-1].
    W2p = np.empty((NK // PACK, 128, PACK * OUT_F), np.float32)
    for j in range(NK // PACK):
        for h in range(PACK):
            W2p[j, :, h * OUT_F : (h + 1) * OUT_F] = W2[CONSUME_ORDER[j * PACK + h]]
    W2 = W2p.reshape(NK // PACK * 128, PACK * OUT_F).astype(np.float16)
    xT = np.ascontiguousarray(x.T)  # [IN_F, BATCH]
    in_maps = [
        {
            "xT": np.ascontiguousarray(xT[:, c * B_LOC : (c + 1) * B_LOC]),
            "w2": W2,
        }
        for c in range(NCORES)
    ]
    return in_maps


def _run(x, spline_weight, base_activation, grid_points, trace=False, cfg=None,
         tmpdir=None):
    from concourse.bass_utils import run_bass_kernel_spmd

    nc = _get_nc(np.asarray(grid_points, np.float32), cfg)
    in_maps = _prep_inputs(x, spline_weight, base_activation, (cfg or CFG)["k_abs"])
    res = run_bass_kernel_spmd(
        nc, in_maps, list(range(NCORES)), trace=trace, tmpdir=tmpdir
    )
    outs = [res.results[c]["out"] for c in range(NCORES)]  # each [OUT_F, B_LOC]
    full = np.concatenate(outs, axis=1)  # [OUT_F, BATCH]
    return np.ascontiguousarray(full.T.astype(np.float32)), res


def kernel(x, spline_weight, base_activation, grid_points):
    out, _ = _run(x, spline_weight, base_activation, grid_points)
    return out
